# revision 2
# baseline (speedup 1.0000x reference)
"""Tensor-parallel x data-parallel SwiGLU MLP (LLaMA-style) on 8 Trainium2
NeuronCores.

Problem: y = (silu(x @ Wg^T) * (x @ Wu^T)) @ Wd^T
  x [2, 2048, 4096] f32, Wg/Wu [11008, 4096] f32, Wd [4096, 11008] f32.

Sharding (2-way tensor-parallel over d_ff x 4-way data-parallel over
tokens): core c takes d_ff half (c % 2) -- 5504 = 43*128 rows, NO padding
needed -- and token quarter (c // 2) -- 1024 tokens, processed in 2 passes
of 512. The host sums the 2 TP partials per token quarter and concatenates
the quarters. This removes the 1376->1408 zero-padding of the 8-way-TP
layout: 8256 matmuls/core instead of 8448 (-2.3%).

Compute is bf16 on the TensorEngine with f32 PSUM accumulation. All DRAM
tensors are pre-laid-out on the host so that every DMA is partition-major
contiguous. Gate/up weight loads ride the sync-engine HWDGE queue, down
weights + outputs the scalar-engine HWDGE queue (so next-pass gate/up
prefetch is never stuck behind down-weight streaming), x loads the gpsimd
SWDGE queue. A block of dummy warmup matmuls trips the PE HAM clock-gate
while the first DMAs land.

kernel(**inputs) -> np.ndarray [2, 2048, 4096] f32.
Set env MLP_KERNEL_TRACE=1 to capture a neuron-profile; the measured
exec_time_ns is then stored in LAST_EXEC_TIME_NS.
"""

import os
import sys
import types

import numpy as np
import ml_dtypes

import concourse.bacc as bacc
import concourse.mybir as mybir
import concourse.tile as tile
from concourse.bass_utils import run_bass_kernel_spmd

P = 128
D = 4096            # d_model
DFF = 11008
NCORES = 8
TPW = 2             # tensor-parallel ways (d_ff split)
DPW = 4             # data-parallel ways (token split)
F = DFF // TPW      # 5504 per core, = 43 * 128 exactly
T = 4096            # total tokens (2 * 2048)
TCORE = T // DPW    # 1024 tokens per core
KD = D // P         # 32 k-subtiles for gate/up
NF = F // P         # 43 f-chunks (k-subtiles for the down proj)
MD = D // P         # 32 output row chunks

TT = 512            # tokens per pass = matmul moving dim / PSUM bank (f32)
NP = TCORE // TT    # 2 passes
XCH = 8             # x DMA chunks per pass
KCH = KD // XCH     # k-subtiles per x chunk
WARMUP = 120

BF16 = mybir.dt.bfloat16
F32 = mybir.dt.float32
NPBF16 = ml_dtypes.bfloat16

LAST_EXEC_TIME_NS = None
_CACHED_NC = None


def _build():
    nc = bacc.Bacc("TRN2", target_bir_lowering=False, debug=False)

    xh = nc.dram_tensor("xh", [NP, P, KD, TT], BF16, kind="ExternalInput")
    wg = nc.dram_tensor("wg", [NF, P, KD, P], BF16, kind="ExternalInput")
    wu = nc.dram_tensor("wu", [NF, P, KD, P], BF16, kind="ExternalInput")
    wd = nc.dram_tensor("wd", [MD, P, NF, P], BF16, kind="ExternalInput")
    y = nc.dram_tensor("y", [MD, P, TCORE], F32, kind="ExternalOutput")

    silu = mybir.ActivationFunctionType.Silu

    with tile.TileContext(nc) as tc:
        with (
            tc.tile_pool(name="xp", bufs=1) as xp,
            tc.tile_pool(name="wgp", bufs=3) as wgp,
            tc.tile_pool(name="wup", bufs=3) as wup,
            tc.tile_pool(name="wdp", bufs=4) as wdp,
            tc.tile_pool(name="hp", bufs=1) as hp,
            tc.tile_pool(name="gp", bufs=4) as gp,
            tc.tile_pool(name="op", bufs=4) as op,
            tc.tile_pool(name="ps", bufs=2, space="PSUM") as ps,
        ):
            # Warm the PE HAM clock-gate while the first DMAs are in
            # flight: dummy matmuls on a zeroed scratch tile into a
            # scratch PSUM bank nobody reads (shares the pg tag's banks).
            wsc = gp.tile([P, 2 * P], BF16, name="wsc", tag="wsc", bufs=1)
            nc.vector.memset(wsc[:], 0.0)
            pw = ps.tile([P, P], F32, name="pw", tag="pg")
            for _ in range(WARMUP):
                nc.tensor.matmul(pw[:], wsc[:, :P], wsc[:, P:],
                                 start=True, stop=True)

            for tp in range(NP):
                # x in XCH per-chunk tiles so gate matmuls can start as
                # soon as chunk 0 + wg[0] land instead of waiting for the
                # full 4 MB. Pass 0 rides the sync queue in a strict
                # startup-critical byte order (wg0, xc0, wu0, xc1..7);
                # later passes load via gpsimd during the previous down
                # phase so they never contend with weight streaming.
                xts = []
                if tp == 0:
                    wgt0 = wgp.tile([P, KD, P], BF16, name="wgt", tag="wgt")
                    nc.sync.dma_start(wgt0[:], wg[0])
                    xts.append(xp.tile([P, KCH, TT], BF16, name="xt0",
                                       tag="xt0"))
                    nc.sync.dma_start(xts[0][:], xh[tp, :, 0:KCH, :])
                    wut0 = wup.tile([P, KD, P], BF16, name="wut", tag="wut")
                    nc.sync.dma_start(wut0[:], wu[0])
                    for c in range(1, XCH):
                        ks = slice(c * KCH, (c + 1) * KCH)
                        xts.append(xp.tile([P, KCH, TT], BF16,
                                           name=f"xt{c}", tag=f"xt{c}"))
                        nc.sync.dma_start(xts[c][:], xh[tp, :, ks, :])
                else:
                    for c in range(XCH):
                        ks = slice(c * KCH, (c + 1) * KCH)
                        xts.append(xp.tile([P, KCH, TT], BF16,
                                           name=f"xt{c}", tag=f"xt{c}"))
                        nc.gpsimd.dma_start(xts[c][:], xh[tp, :, ks, :])
                ht = hp.tile([P, NF, TT], BF16, name="ht", tag="ht")
                for fi in range(NF):
                    if tp == 0 and fi == 0:
                        wgt, wut = wgt0, wut0
                    else:
                        wgt = wgp.tile([P, KD, P], BF16, name="wgt",
                                       tag="wgt")
                        nc.sync.dma_start(wgt[:], wg[fi])
                        wut = wup.tile([P, KD, P], BF16, name="wut",
                                       tag="wut")
                        nc.sync.dma_start(wut[:], wu[fi])
                    pg = ps.tile([P, TT], F32, name="pg", tag="pg")
                    for k in range(KD):
                        nc.tensor.matmul(pg[:], wgt[:, k, :],
                                         xts[k // KCH][:, k % KCH, :],
                                         start=(k == 0), stop=(k == KD - 1))
                    pu = ps.tile([P, TT], F32, name="pu", tag="pu")
                    for k in range(KD):
                        nc.tensor.matmul(pu[:], wut[:, k, :],
                                         xts[k // KCH][:, k % KCH, :],
                                         start=(k == 0), stop=(k == KD - 1))
                    gt = gp.tile([P, TT], BF16, name="gt", tag="gt")
                    nc.scalar.activation(gt[:], pg[:], silu)
                    nc.vector.tensor_mul(ht[:, fi, :], pu[:], gt[:])
                for mi in range(MD):
                    wdt = wdp.tile([P, NF, P], BF16, name="wdt", tag="wdt")
                    nc.sync.dma_start(wdt[:], wd[mi])
                    py = ps.tile([P, TT], F32, name="py", tag="py", bufs=4)
                    for k in range(NF):
                        nc.tensor.matmul(py[:], wdt[:, k, :], ht[:, k, :],
                                         start=(k == 0), stop=(k == NF - 1))
                    ot = op.tile([P, TT], F32, name="ot", tag="ot")
                    # Evictions alternate ACT/DVE and are emitted at
                    # high scheduler priority: a single eviction FIFO
                    # gets head-of-line blocked behind silu ops that
                    # wait on the PE, which in turn waits on the PSUM
                    # bank the eviction would free.
                    with tc.high_priority():
                        if mi % 2 == 0:
                            nc.scalar.copy(ot[:], py[:])
                        else:
                            nc.vector.tensor_copy(ot[:], py[:])
                        off = tp * TT
                        nc.scalar.dma_start(y[mi, :, off:off + TT], ot[:])

    nc.compile()
    return nc


def _prep_inputs(x, W_gate, W_up, W_down):
    xf = np.ascontiguousarray(np.asarray(x, dtype=np.float32)).reshape(T, D)
    # xh_q[tp, p, k, t] = x[q*TCORE + tp*TT + t, k*128 + p]
    xq = [
        np.ascontiguousarray(
            xf[q * TCORE:(q + 1) * TCORE]
            .reshape(NP, TT, KD, P).transpose(0, 3, 2, 1)).astype(NPBF16)
        for q in range(DPW)
    ]

    Wg = np.asarray(W_gate, dtype=np.float32)
    Wu = np.asarray(W_up, dtype=np.float32)
    Wd = np.asarray(W_down, dtype=np.float32)

    wgh, wuh, wdh = [], [], []
    for fh in range(TPW):
        fs = fh * F
        # wg[fi, p, k, j] = Wg[fs + fi*128 + j, k*128 + p]
        wgh.append(np.ascontiguousarray(
            Wg[fs:fs + F].reshape(NF, P, KD, P).transpose(0, 3, 2, 1))
            .astype(NPBF16))
        wuh.append(np.ascontiguousarray(
            Wu[fs:fs + F].reshape(NF, P, KD, P).transpose(0, 3, 2, 1))
            .astype(NPBF16))
        # wd[mi, p, k, j] = Wd[mi*128 + j, fs + k*128 + p]
        wdh.append(np.ascontiguousarray(
            Wd[:, fs:fs + F].reshape(MD, P, NF, P).transpose(0, 3, 2, 1))
            .astype(NPBF16))

    in_maps = []
    for c in range(NCORES):
        fh, q = c % TPW, c // TPW
        in_maps.append(
            {"xh": xq[q], "wg": wgh[fh], "wu": wuh[fh], "wd": wdh[fh]})
    return in_maps


def _install_ntff_shim():
    """antenv.axon_hooks is missing from some images; register an
    equivalent module so trace=True can capture NTFF profiles."""
    try:
        import antenv.axon_hooks  # noqa: F401
        return True
    except ImportError:
        pass
    try:
        import antenv
        from trn_agent_boot.trn_boot import _ntff_profile_via_ctypes
        hook = _ntff_profile_via_ctypes('/opt/axon/libaxon_pjrt.so')
        mod = types.ModuleType("antenv.axon_hooks")
        mod._hook = hook
        mod.get_axon_ntff_profile_hook = lambda: mod._hook

        def set_axon_ntff_profile_hook(h):
            mod._hook = h

        mod.set_axon_ntff_profile_hook = set_axon_ntff_profile_hook
        sys.modules["antenv.axon_hooks"] = mod
        antenv.axon_hooks = mod
        return True
    except Exception:
        return False


def kernel(x, W_gate, W_up, W_down):
    global LAST_EXEC_TIME_NS, _CACHED_NC
    if _CACHED_NC is None:
        _CACHED_NC = _build()
    nc = _CACHED_NC

    in_maps = _prep_inputs(x, W_gate, W_up, W_down)

    trace = os.environ.get("MLP_KERNEL_TRACE", "0") == "1"
    if trace:
        trace = _install_ntff_shim()

    res = run_bass_kernel_spmd(nc, in_maps, list(range(NCORES)), trace=trace)
    LAST_EXEC_TIME_NS = res.exec_time_ns

    # Per token quarter q, sum the 2 TP partials: acc[mi, p, t] =
    # y^T[mi*128+p, q*1024 + t]; then transpose back and concatenate.
    yout = np.empty((T, D), np.float32)
    for q in range(DPW):
        acc = res.results[TPW * q]["y"].astype(np.float32, copy=True)
        for fh in range(1, TPW):
            acc += res.results[TPW * q + fh]["y"]
        yout[q * TCORE:(q + 1) * TCORE] = (
            acc.transpose(2, 0, 1).reshape(TCORE, D))
    return yout.reshape(2, 2048, D)


# revision 3
# speedup vs baseline: 1.1993x; 1.1993x over previous
"""Tensor-parallel x data-parallel SwiGLU MLP (LLaMA-style) on 8 Trainium2
NeuronCores.

Problem: y = (silu(x @ Wg^T) * (x @ Wu^T)) @ Wd^T
  x [2, 2048, 4096] f32, Wg/Wu [11008, 4096] f32, Wd [4096, 11008] f32.

Sharding (2-way tensor-parallel over d_ff x 4-way data-parallel over
tokens): core c takes d_ff half (c % 2) -- 5504 = 43*128 rows, NO padding
needed -- and token quarter (c // 2) -- 1024 tokens, processed in 2 passes
of 512. The host sums the 2 TP partials per token quarter and concatenates
the quarters. This removes the 1376->1408 zero-padding of the 8-way-TP
layout: 8256 matmuls/core instead of 8448 (-2.3%).

Compute is bf16 on the TensorEngine with f32 PSUM accumulation. All DRAM
tensors are pre-laid-out on the host so that every DMA is partition-major
contiguous. Gate/up weight loads ride the sync-engine HWDGE queue, down
weights + outputs the scalar-engine HWDGE queue (so next-pass gate/up
prefetch is never stuck behind down-weight streaming), x loads the gpsimd
SWDGE queue. A block of dummy warmup matmuls trips the PE HAM clock-gate
while the first DMAs land.

kernel(**inputs) -> np.ndarray [2, 2048, 4096] f32.
Set env MLP_KERNEL_TRACE=1 to capture a neuron-profile; the measured
exec_time_ns is then stored in LAST_EXEC_TIME_NS.
"""

import os
import sys
import types

import numpy as np
import ml_dtypes

import concourse.bacc as bacc
import concourse.mybir as mybir
import concourse.tile as tile
from concourse.bass_utils import run_bass_kernel_spmd

P = 128
D = 4096            # d_model
DFF = 11008
NCORES = 8
TPW = 2             # tensor-parallel ways (d_ff split)
DPW = 4             # data-parallel ways (token split)
F = DFF // TPW      # 5504 per core, = 43 * 128 exactly
T = 4096            # total tokens (2 * 2048)
TCORE = T // DPW    # 1024 tokens per core
KD = D // P         # 32 k-subtiles for gate/up
NF = F // P         # 43 f-chunks (k-subtiles for the down proj)
MD = D // P         # 32 output row chunks

TT = 512            # tokens per pass = matmul moving dim / PSUM bank (f32)
NP = TCORE // TT    # 2 passes
XCH = 8             # x DMA chunks per pass
KCH = KD // XCH     # k-subtiles per x chunk
WARMUP = 120

BF16 = mybir.dt.bfloat16
F32 = mybir.dt.float32
NPBF16 = ml_dtypes.bfloat16

LAST_EXEC_TIME_NS = None
_CACHED_NC = None


def _build():
    nc = bacc.Bacc("TRN2", target_bir_lowering=False, debug=False)

    xh = nc.dram_tensor("xh", [NP, P, KD, TT], BF16, kind="ExternalInput")
    wg = nc.dram_tensor("wg", [NF, P, KD, P], BF16, kind="ExternalInput")
    wu = nc.dram_tensor("wu", [NF, P, KD, P], BF16, kind="ExternalInput")
    wd = nc.dram_tensor("wd", [MD, P, NF, P], BF16, kind="ExternalInput")
    y = nc.dram_tensor("y", [MD, P, TCORE], F32, kind="ExternalOutput")

    silu = mybir.ActivationFunctionType.Silu

    with tile.TileContext(nc) as tc:
        with (
            tc.tile_pool(name="xp", bufs=1) as xp,
            tc.tile_pool(name="wgp", bufs=3) as wgp,
            tc.tile_pool(name="wup", bufs=3) as wup,
            tc.tile_pool(name="wdp", bufs=4) as wdp,
            tc.tile_pool(name="hp", bufs=1) as hp,
            tc.tile_pool(name="gp", bufs=4) as gp,
            tc.tile_pool(name="op", bufs=4) as op,
            tc.tile_pool(name="ps", bufs=2, space="PSUM") as ps,
        ):
            # Warm the PE HAM clock-gate while the first DMAs are in
            # flight: dummy matmuls on a zeroed scratch tile into a
            # scratch PSUM bank nobody reads (shares the pg tag's banks).
            wsc = gp.tile([P, 2 * P], BF16, name="wsc", tag="wsc", bufs=1)
            nc.vector.memset(wsc[:], 0.0)
            pw = ps.tile([P, P], F32, name="pw", tag="pg")
            for _ in range(WARMUP):
                nc.tensor.matmul(pw[:], wsc[:, :P], wsc[:, P:],
                                 start=True, stop=True)

            for tp in range(NP):
                # x in XCH per-chunk tiles so gate matmuls can start as
                # soon as chunk 0 + wg[0] land instead of waiting for the
                # full 4 MB. Pass 0 rides the sync queue in a strict
                # startup-critical byte order (wg0, xc0, wu0, xc1..7);
                # later passes load via gpsimd during the previous down
                # phase so they never contend with weight streaming.
                xts = []
                if tp == 0:
                    wgt0 = wgp.tile([P, KD, P], BF16, name="wgt", tag="wgt")
                    nc.sync.dma_start(wgt0[:], wg[0])
                    for c in range(2):
                        ks = slice(c * KCH, (c + 1) * KCH)
                        xts.append(xp.tile([P, KCH, TT], BF16,
                                           name=f"xt{c}", tag=f"xt{c}"))
                        nc.sync.dma_start(xts[c][:], xh[tp, :, ks, :])
                    # wu[0] is first consumed ~7us after the gate group
                    # starts, so it queues behind two x chunks: the paced
                    # gate matmuls never starve waiting on it.
                    wut0 = wup.tile([P, KD, P], BF16, name="wut", tag="wut")
                    nc.sync.dma_start(wut0[:], wu[0])
                    for c in range(2, XCH):
                        ks = slice(c * KCH, (c + 1) * KCH)
                        xts.append(xp.tile([P, KCH, TT], BF16,
                                           name=f"xt{c}", tag=f"xt{c}"))
                        nc.sync.dma_start(xts[c][:], xh[tp, :, ks, :])
                else:
                    for c in range(XCH):
                        ks = slice(c * KCH, (c + 1) * KCH)
                        xts.append(xp.tile([P, KCH, TT], BF16,
                                           name=f"xt{c}", tag=f"xt{c}"))
                        nc.gpsimd.dma_start(xts[c][:], xh[tp, :, ks, :])
                ht = hp.tile([P, NF, TT], BF16, name="ht", tag="ht")
                for fi in range(NF):
                    if tp == 0 and fi == 0:
                        wgt, wut = wgt0, wut0
                    else:
                        wgt = wgp.tile([P, KD, P], BF16, name="wgt",
                                       tag="wgt")
                        nc.sync.dma_start(wgt[:], wg[fi])
                        wut = wup.tile([P, KD, P], BF16, name="wut",
                                       tag="wut")
                        nc.sync.dma_start(wut[:], wu[fi])
                    pg = ps.tile([P, TT], F32, name="pg", tag="pg")
                    for k in range(KD):
                        nc.tensor.matmul(pg[:], wgt[:, k, :],
                                         xts[k // KCH][:, k % KCH, :],
                                         start=(k == 0), stop=(k == KD - 1))
                    pu = ps.tile([P, TT], F32, name="pu", tag="pu")
                    for k in range(KD):
                        nc.tensor.matmul(pu[:], wut[:, k, :],
                                         xts[k // KCH][:, k % KCH, :],
                                         start=(k == 0), stop=(k == KD - 1))
                    gt = gp.tile([P, TT], BF16, name="gt", tag="gt")
                    nc.scalar.activation(gt[:], pg[:], silu)
                    nc.vector.tensor_mul(ht[:, fi, :], pu[:], gt[:])
                for mi in range(MD):
                    wdt = wdp.tile([P, NF, P], BF16, name="wdt", tag="wdt")
                    nc.sync.dma_start(wdt[:], wd[mi])
                    py = ps.tile([P, TT], F32, name="py", tag="py", bufs=4)
                    for k in range(NF):
                        nc.tensor.matmul(py[:], wdt[:, k, :], ht[:, k, :],
                                         start=(k == 0), stop=(k == NF - 1))
                    ot = op.tile([P, TT], F32, name="ot", tag="ot")
                    # Evictions alternate ACT/DVE and are emitted at
                    # high scheduler priority: a single eviction FIFO
                    # gets head-of-line blocked behind silu ops that
                    # wait on the PE, which in turn waits on the PSUM
                    # bank the eviction would free.
                    with tc.high_priority():
                        if mi % 2 == 0:
                            nc.scalar.copy(ot[:], py[:])
                        else:
                            nc.vector.tensor_copy(ot[:], py[:])
                        off = tp * TT
                        nc.scalar.dma_start(y[mi, :, off:off + TT], ot[:])

    nc.compile()
    return nc


def _prep_inputs(x, W_gate, W_up, W_down):
    xf = np.ascontiguousarray(np.asarray(x, dtype=np.float32)).reshape(T, D)
    # xh_q[tp, p, k, t] = x[q*TCORE + tp*TT + t, k*128 + p]
    xq = [
        np.ascontiguousarray(
            xf[q * TCORE:(q + 1) * TCORE]
            .reshape(NP, TT, KD, P).transpose(0, 3, 2, 1)).astype(NPBF16)
        for q in range(DPW)
    ]

    Wg = np.asarray(W_gate, dtype=np.float32)
    Wu = np.asarray(W_up, dtype=np.float32)
    Wd = np.asarray(W_down, dtype=np.float32)

    wgh, wuh, wdh = [], [], []
    for fh in range(TPW):
        fs = fh * F
        # wg[fi, p, k, j] = Wg[fs + fi*128 + j, k*128 + p]
        wgh.append(np.ascontiguousarray(
            Wg[fs:fs + F].reshape(NF, P, KD, P).transpose(0, 3, 2, 1))
            .astype(NPBF16))
        wuh.append(np.ascontiguousarray(
            Wu[fs:fs + F].reshape(NF, P, KD, P).transpose(0, 3, 2, 1))
            .astype(NPBF16))
        # wd[mi, p, k, j] = Wd[mi*128 + j, fs + k*128 + p]
        wdh.append(np.ascontiguousarray(
            Wd[:, fs:fs + F].reshape(MD, P, NF, P).transpose(0, 3, 2, 1))
            .astype(NPBF16))

    in_maps = []
    for c in range(NCORES):
        fh, q = c % TPW, c // TPW
        in_maps.append(
            {"xh": xq[q], "wg": wgh[fh], "wu": wuh[fh], "wd": wdh[fh]})
    return in_maps


def _install_ntff_shim():
    """antenv.axon_hooks is missing from some images; register an
    equivalent module so trace=True can capture NTFF profiles."""
    try:
        import antenv.axon_hooks  # noqa: F401
        return True
    except ImportError:
        pass
    try:
        import antenv
        from trn_agent_boot.trn_boot import _ntff_profile_via_ctypes
        hook = _ntff_profile_via_ctypes('/opt/axon/libaxon_pjrt.so')
        mod = types.ModuleType("antenv.axon_hooks")
        mod._hook = hook
        mod.get_axon_ntff_profile_hook = lambda: mod._hook

        def set_axon_ntff_profile_hook(h):
            mod._hook = h

        mod.set_axon_ntff_profile_hook = set_axon_ntff_profile_hook
        sys.modules["antenv.axon_hooks"] = mod
        antenv.axon_hooks = mod
        return True
    except Exception:
        return False


def kernel(x, W_gate, W_up, W_down):
    global LAST_EXEC_TIME_NS, _CACHED_NC
    if _CACHED_NC is None:
        _CACHED_NC = _build()
    nc = _CACHED_NC

    in_maps = _prep_inputs(x, W_gate, W_up, W_down)

    trace = os.environ.get("MLP_KERNEL_TRACE", "0") == "1"
    if trace:
        trace = _install_ntff_shim()

    res = run_bass_kernel_spmd(nc, in_maps, list(range(NCORES)), trace=trace)
    LAST_EXEC_TIME_NS = res.exec_time_ns

    # Per token quarter q, sum the 2 TP partials: acc[mi, p, t] =
    # y^T[mi*128+p, q*1024 + t]; then transpose back and concatenate.
    yout = np.empty((T, D), np.float32)
    for q in range(DPW):
        acc = res.results[TPW * q]["y"].astype(np.float32, copy=True)
        for fh in range(1, TPW):
            acc += res.results[TPW * q + fh]["y"]
        yout[q * TCORE:(q + 1) * TCORE] = (
            acc.transpose(2, 0, 1).reshape(TCORE, D))
    return yout.reshape(2, 2048, D)
